# revision 20
# baseline (speedup 1.0000x reference)
"""Dense-recompute distributed Trainium2 kernel for:
    out = x.at[target_idx].set(relu(x[arg_idx] @ W + b))

N=2097152 rows x D=64, K=1048576 gathered/scattered rows, 8 NeuronCores.

Strategy v3 (vs the v2 gather kernel at ~1.38 ms): random-row gathers are
descriptor-rate-bound (~9 ns/row/core), so instead every core streams its own
contiguous 33.5 MB slice of a pre-staged bf16 copy of x ONCE and computes
f(row) = relu(row @ W + b) for ALL of its rows densely - sequential DMA only:

  per 16384-row chunk ([128, 8192] bf16 tile, partition-major):
    - load on the sync queue; pass-through store (unchanged bytes) on the
      scalar queue
    - PE-transposes 128-col slices into PSUM (feat-on-partition, two windows
      stacked), DVE copies PSUM->SBUF, ONE K=128 matmul per 512 rows against
      the block-diagonal [[W,0],[0,W]] (operands must be partition-0 based:
      offset-64 matmul operands wedge this hardware), ACT relu(+bias) writes
      the bf16 flush tile directly in y^T layout (no back-transpose)
    - F flush store alternates sync/scalar queues

Outputs per core: [F stream y^T-layout; pass-through row-major], both bf16
(gate is rel_err < 2e-2; bf16 lands ~2e-3). All output bytes are produced on
device; the host only routes: pass-through rows via an identity row gather,
target rows via a vectorized element gather from the y^T layout.

Per-core traffic 100.5 MB at the measured ~390 GB/s/core. Fully DMA-bound:
a pure-DMA ablation of the same 3-stream pattern (parts='pt3') times within
~8 us of the full kernel, so the whole compute pipeline hides under the DMA.
Measured on 8 axon-tunneled trn2 cores via repeat-16-in-NEFF chain slope
(warmup pair discarded): ~310-320 us/exec, rel err 2.14e-3, vs the 1.48 ms
v2 gather baseline -- ~4.7x. Adopted: '_f2' split F stores + '_io' per-chunk
queue interleave (+6 us in head-to-head). Rejected by measurement: finer chunks (_ch8), queue
interleave (_io), gpsimd F-stores (_fg), direct DRAM->DRAM pass-through
(_pd, 100 us WORSE: few-descriptor D2D parallelizes poorly), F compaction
(one-hot mask gen on DVE costs ~350 us to save ~60 us of DMA), fp8 F stream
(norm rel err ~2.5e-2 exceeds the 2e-2 gate).
"""

import numpy as np
import ml_dtypes

import jax
import jax.numpy as jnp
from jax.sharding import Mesh, PartitionSpec, NamedSharding
from jax.experimental.shard_map import shard_map

import concourse.bass as bass
import concourse.bacc as bacc
import concourse.mybir as mybir
import concourse.bass2jax as bass2jax
from concourse.tile import TileContext

bf16 = ml_dtypes.bfloat16

N = 2097152
D = 64
K = 1048576
NC = 8
N8 = N // NC          # 262144 rows per core
CH = 16384            # rows per chunk
C = CH // 128         # 128 col-blocks (rows per partition) per chunk
NCH = N8 // CH        # 16 chunks per core
NG = C // 8           # 16 groups of 8 col-blocks (512 cols) per chunk

_CAP = 1  # walrus: one semaphore wait per instruction


def _split_excess_waits(nc):
    for f in nc.m.functions:
        for bb in f.blocks:
            insts = list(bb.instructions)
            out = []
            changed = False
            for inst in insts:
                si = inst.sync_info
                if si is not None and len(si.on_wait) > _CAP:
                    waits = list(si.on_wait)
                    head, tail = waits[:-_CAP], waits[-_CAP:]
                    for i in range(0, len(head), _CAP):
                        nop = mybir.InstNoOp(
                            name=f"waitsplit_{nc.next_id()}", ins=[], outs=[])
                        nop.engine = inst.engine
                        nop.sync_info = mybir.SyncInfo(
                            on_wait=head[i:i + _CAP], on_update=[])
                        out.append(nop)
                    si.on_wait = tail
                    inst.sync_info = si
                    changed = True
                out.append(inst)
            if changed:
                bb.instructions = out


PARTS = ["full", "pt", "comp", "engines", "comp1", "comp2", "empty", "pt3"]
BUILD_SALT = 2  # bump on any kernel change: the NEFF cache hashes HLO shapes,
                # not the embedded BIR, so same-shape rebuilds go stale


def build_nc(repeat=1, parts="full", split_waits=True):
    """parts base: 'full' | 'pt' | 'comp' (no PT store) | 'engines' (1 load,
    all compute) | 'comp1' (transpose+copy only) | 'comp2' (no back-transpose).
    Suffixes: '_fg' F stores on gpsimd (SWDGE) queue; '_bm' bias via K=1 matmul
    instead of ACT bias AP."""
    f_gp = "_fg" in parts
    bias_mm = "_bm" in parts
    b4 = "_b4" in parts   # deeper pool buffering
    c2 = "_c2" in parts   # split psum->sbuf copies in halves for latency hiding
    io = "_io" in parts   # interleave load/PT/F queue assignment per chunk
    pd = "_pd" in parts   # pass-through as direct DRAM->DRAM (no SBUF dep)
    f2 = "_f2" in parts   # F store in two halves (earlier queue feed)
    ch = CH // 2 if "_ch8" in parts else (CH * 2 if "_ch32" in parts else CH)
    c_w = ch // 128       # col-blocks per chunk
    ng = c_w // 8         # 512-col groups per chunk
    nch = N8 // ch
    nc = bacc.Bacc(num_swdge_queues=1)
    nc.declare_dram_parameter(
        "dum", [1, 1 + BUILD_SALT * 16 + PARTS.index(parts.split("_")[0])],
        mybir.dt.float32, isOutput=False)
    xb = nc.declare_dram_parameter("xb", [N8, D], mybir.dt.bfloat16, isOutput=False)
    wsb = nc.declare_dram_parameter("wsb", [128, 128], mybir.dt.bfloat16, isOutput=False)
    b2d = nc.declare_dram_parameter("b2d", [128, 1], mybir.dt.float32, isOutput=False)
    brd = nc.declare_dram_parameter("brd", [1, 128], mybir.dt.bfloat16, isOutput=False)
    oned = nc.declare_dram_parameter("oned", [1, 512], mybir.dt.bfloat16, isOutput=False)
    idd = nc.declare_dram_parameter("idd", [128, 128], mybir.dt.bfloat16, isOutput=False)
    out = nc.declare_dram_parameter("out", [2 * N8, D], mybir.dt.bfloat16, isOutput=True)

    base = parts.split("_")[0]
    do_pt = base in ("full", "pt", "pt3")
    do_comp = base not in ("pt", "pt3", "empty")
    one_load = base == "engines"
    yt_mode = "_yt" in parts  # F stays in y^T layout; no back-transposes
    # comp1: stop after transpose+copy (store xts); comp2: stop after relu
    # (store yt); 3 = full pipeline
    stage = {"comp1": 1, "comp2": 2}.get(base, 3)

    with TileContext(nc) as tc:
        with (
            tc.tile_pool(name="wt", bufs=1) as wpool,
            tc.tile_pool(name="x", bufs=3 if b4 else 2) as xpool,
            tc.tile_pool(name="xt", bufs=4 if b4 else 3) as xtpool,
            tc.tile_pool(name="yt", bufs=3) as ytpool,
            tc.tile_pool(name="ft", bufs=3 if b4 else 2) as fpool,
            tc.tile_pool(name="pT", bufs=4 if b4 else 2, space="PSUM") as pTpool,
            tc.tile_pool(name="pY", bufs=3 if b4 else 2, space="PSUM") as pYpool,
            tc.tile_pool(name="pR", bufs=2, space="PSUM") as pRpool,
        ):
            wt = wpool.tile([128, 128], mybir.dt.bfloat16, tag="wt")
            bt = wpool.tile([128, 1], mybir.dt.float32, tag="bt")
            brt = wpool.tile([1, 128], mybir.dt.bfloat16, tag="brt")
            onet = wpool.tile([1, 512], mybir.dt.bfloat16, tag="onet")
            idt = wpool.tile([128, 128], mybir.dt.bfloat16, tag="idt")
            nc.sync.dma_start(out=wt[:], in_=wsb[:, :])
            nc.sync.dma_start(out=bt[:], in_=b2d[:, :])
            nc.sync.dma_start(out=brt[:], in_=brd[:, :])
            nc.sync.dma_start(out=onet[:], in_=oned[:, :])
            nc.sync.dma_start(out=idt[:], in_=idd[:, :])

            for rep in range(repeat):
                if base == "empty":
                    et = xpool.tile([128, D], mybir.dt.bfloat16, tag="e",
                                    name=f"e_{rep}")
                    nc.sync.dma_start(
                        out=et[:],
                        in_=xb[0:128, :].rearrange("(p c) d -> p (c d)", p=128))
                    nc.scalar.dma_start(
                        out=out[0:128, :].rearrange("(p c) d -> p (c d)", p=128),
                        in_=et[:])
                    continue
                for c in range(nch):
                    s = c * ch
                    ld_eng = (nc.sync if (not io or c % 2 == 0) else nc.scalar)
                    pt_eng = (nc.scalar if (not io or c % 2 == 0) else nc.sync)
                    xt_ = xpool.tile([128, c_w * D], mybir.dt.bfloat16, tag="x",
                                     name=f"x_{rep}_{c}")
                    if not one_load or c == 0:
                        ld_eng.dma_start(
                            out=xt_[:],
                            in_=xb[s:s + ch, :].rearrange("(p c) d -> p (c d)", p=128))
                    if do_pt:
                        if pd:
                            # direct DRAM->DRAM identity copy: same bytes, but
                            # independent of the SBUF load
                            pt_eng.dma_start(
                                out=out[N8 + s:N8 + s + ch, :],
                                in_=xb[s:s + ch, :])
                        else:
                            pt_eng.dma_start(
                                out=out[N8 + s:N8 + s + ch, :].rearrange(
                                    "(p c) d -> p (c d)", p=128),
                                in_=xt_[:])
                    if base == "pt3":
                        feng = nc.sync if c % 2 == 0 else nc.scalar
                        feng.dma_start(
                            out=out[s:s + ch, :].rearrange("(p c) d -> p (c d)", p=128),
                            in_=xt_[:])
                        continue
                    if not do_comp:
                        continue
                    ft = fpool.tile([128, c_w * D], mybir.dt.bfloat16, tag="ft",
                                    name=f"ft_{rep}_{c}")
                    for g in range(ng):
                        c0 = g * 512
                        pT = pTpool.tile([128, 512], mybir.dt.bfloat16, tag="pT",
                                         name=f"pT_{rep}_{c}_{g}")
                        for q in range(4):
                            nc.tensor.transpose(
                                pT[:, q * 128:(q + 1) * 128],
                                xt_[:, c0 + q * 128:c0 + (q + 1) * 128],
                                idt[:])
                        xts = xtpool.tile([128, 512], mybir.dt.bfloat16, tag="xts",
                                          name=f"xts_{rep}_{c}_{g}")
                        if c2:
                            nc.vector.tensor_copy(out=xts[:, 0:256], in_=pT[:, 0:256])
                            nc.vector.tensor_copy(out=xts[:, 256:512], in_=pT[:, 256:512])
                        else:
                            nc.vector.tensor_copy(out=xts[:], in_=pT[:])
                        if stage == 1:
                            nc.vector.tensor_copy(out=ft[:, c0:c0 + 512], in_=xts[:])
                            continue
                        pY = pYpool.tile([128, 512], mybir.dt.float32, tag="pY",
                                         name=f"pY_{rep}_{c}_{g}")
                        # single K=128 matmul: wt is block-diag [[W,0],[0,W]],
                        # so both window halves compute at once with all
                        # operands partition-0 based (offset-64 operands wedge
                        # this hardware)
                        if bias_mm:
                            nc.tensor.matmul(pY[:], brt[0:1, :], onet[0:1, :],
                                             start=True, stop=False)
                            nc.tensor.matmul(pY[:], wt[:, :], xts[:, :],
                                             start=False, stop=True)
                        else:
                            nc.tensor.matmul(pY[:], wt[:, :], xts[:, :],
                                             start=True, stop=True)
                        relu_kw = {} if bias_mm else {"bias": bt[:, 0:1]}
                        if yt_mode:
                            # relu writes the flush tile directly (y^T layout)
                            nc.scalar.activation(
                                ft[:, c0:c0 + 512], pY[:],
                                mybir.ActivationFunctionType.Relu, **relu_kw)
                            continue
                        yt = ytpool.tile([128, 512], mybir.dt.bfloat16, tag="yt",
                                         name=f"yt_{rep}_{c}_{g}")
                        nc.scalar.activation(
                            yt[:], pY[:], mybir.ActivationFunctionType.Relu,
                            **relu_kw)
                        if stage == 2:
                            nc.vector.tensor_copy(out=ft[:, c0:c0 + 512], in_=yt[:])
                            continue
                        pR = pRpool.tile([128, 512], mybir.dt.bfloat16, tag="pR",
                                         name=f"pR_{rep}_{c}_{g}")
                        for q in range(4):
                            nc.tensor.transpose(
                                pR[:, q * 128:(q + 1) * 128],
                                yt[:, q * 128:(q + 1) * 128],
                                idt[:])
                        nc.vector.tensor_copy(out=ft[:, c0:c0 + 512], in_=pR[:])
                    # F store: gpsimd queue, or alternate sync/scalar
                    if f_gp:
                        feng = nc.gpsimd
                    else:
                        feng = nc.sync if c % 2 == 0 else nc.scalar
                    fap = out[s:s + ch, :].rearrange("(p c) d -> p (c d)", p=128)
                    if f2:
                        half = c_w * D // 2
                        feng.dma_start(out=fap[:, 0:half], in_=ft[:, 0:half])
                        feng.dma_start(out=fap[:, half:], in_=ft[:, half:])
                    else:
                        feng.dma_start(out=fap, in_=ft[:])
    nc.compile()
    if split_waits:
        _split_excess_waits(nc)
    return nc


_CACHE = {}


def _get_callable(repeat=1, parts="full"):
    key = f"fn_{repeat}_{parts}"
    if key in _CACHE:
        return _CACHE[key]
    bass2jax.install_neuronx_cc_hook()
    nc = build_nc(repeat, parts)

    pname = nc.partition_id_tensor.name if nc.partition_id_tensor else None
    in_names, out_names, out_avals = [], [], []
    for alloc in nc.m.functions[0].allocations:
        if not isinstance(alloc, mybir.MemoryLocationSet):
            continue
        name = alloc.memorylocations[0].name
        if alloc.kind == "ExternalInput":
            if name != pname:
                in_names.append(name)
        elif alloc.kind == "ExternalOutput":
            out_names.append(name)
            out_avals.append(
                jax.core.ShapedArray(tuple(alloc.tensor_shape), mybir.dt.np(alloc.dtype)))
    assert in_names[0] == "dum", in_names
    dum_shape = None
    for alloc in nc.m.functions[0].allocations:
        if (isinstance(alloc, mybir.MemoryLocationSet)
                and alloc.memorylocations[0].name == "dum"):
            dum_shape = tuple(alloc.tensor_shape)
    n_params = len(in_names)
    all_in = list(in_names) + list(out_names)
    if pname is not None:
        all_in.append(pname)

    def _body(*args):
        operands = list(args)
        if pname is not None:
            operands.append(bass2jax.partition_id_tensor())
        outs = bass2jax._bass_exec_p.bind(
            *operands,
            out_avals=tuple(out_avals),
            in_names=tuple(all_in),
            out_names=tuple(out_names),
            lowering_input_output_aliases=(),
            sim_require_finite=True,
            sim_require_nnan=True,
            nc=nc)
        return tuple(outs)

    devices = jax.devices()[:NC]
    mesh = Mesh(np.asarray(devices), ("core",))
    repl_names = {"wsb", "b2d", "brd", "oned", "idd", "dum"}
    in_specs = tuple(
        PartitionSpec(None) if n in repl_names else PartitionSpec("core")
        for n in in_names) + (PartitionSpec("core"),) * len(out_names)
    out_specs = (PartitionSpec("core"),) * len(out_names)
    fn = jax.jit(
        shard_map(_body, mesh=mesh, in_specs=in_specs, out_specs=out_specs,
                  check_rep=False),
        donate_argnums=tuple(range(n_params, n_params + len(out_names))),
        keep_unused=True)
    dum = jax.device_put(np.zeros(dum_shape, np.float32),
                         NamedSharding(mesh, PartitionSpec(None)))
    _CACHE[key] = (fn, in_names, out_names, mesh, dum)
    return _CACHE[key]


BEST = "full_yt_b4_f2_io"  # parts string used by kernel() and the test harness


def _assemble_rowmajor(res, arg, tgt):
    i = np.arange(N, dtype=np.int64)
    inv = (i // N8) * (2 * N8) + N8 + (i % N8)
    inv[tgt] = (arg // N8) * (2 * N8) + (arg % N8)
    return res[inv].astype(np.float32)


def _assemble_yt(res, arg, tgt):
    """F region is y^T-layout: feat f of in-chunk row rr=(p2*?; see build) lives
    at ft[64*(j%2)+f, (j//8)*512 + ((j%8)//2)*128 + p2], j=rr%128, p2=rr//128."""
    i = np.arange(N, dtype=np.int64)
    inv0 = (i // N8) * (2 * N8) + N8 + (i % N8)
    out = res[inv0].astype(np.float32)
    a = arg
    core = a // N8
    rr0 = a % N8
    c = rr0 // CH
    rr = rr0 % CH
    p2 = rr // C
    j = rr % C
    u = (j // 8) * 512 + ((j % 8) // 2) * 128 + p2
    m0 = 64 * (j % 2)
    base = core * (2 * N8) + c * CH + m0 * C + (u // 64)
    row_idx = base[:, None] + (np.arange(64, dtype=np.int64) * C)[None, :]
    col_idx = np.broadcast_to((u % 64)[:, None], row_idx.shape)
    vals = res[row_idx, col_idx].astype(np.float32)
    out[tgt] = vals
    return out


def assemble(res, arg, tgt, parts=None):
    parts = BEST if parts is None else parts
    if "_yt" in parts:
        return _assemble_yt(res, arg, tgt)
    return _assemble_rowmajor(res, arg, tgt)


def prepare(x, W, b, arg_idx, target_idx):
    x = np.asarray(x, dtype=np.float32)
    W = np.asarray(W, dtype=np.float32)
    b = np.asarray(b, dtype=np.float32)
    arg = np.asarray(arg_idx, dtype=np.int64)
    tgt = np.asarray(target_idx, dtype=np.int64)

    xb = x.astype(bf16)
    Wb = W.astype(bf16)
    wsb = np.zeros((128, 128), dtype=bf16)
    wsb[0:64, 0:64] = Wb
    wsb[64:128, 64:128] = Wb
    b2d = np.concatenate([b, b]).reshape(128, 1).astype(np.float32)
    brd = np.concatenate([b, b]).reshape(1, 128).astype(bf16)
    oned = np.ones((1, 512), dtype=bf16)
    idd = np.eye(128, dtype=bf16)

    # inverse map: PT region (identity) for everyone, F region for targets
    i = np.arange(N, dtype=np.int64)
    inv = (i // N8) * (2 * N8) + N8 + (i % N8)
    inv[tgt] = (arg // N8) * (2 * N8) + (arg % N8)

    fn, in_names, out_names, mesh, dum = _get_callable(1, BEST)
    repl = NamedSharding(mesh, PartitionSpec(None))
    shard = NamedSharding(mesh, PartitionSpec("core"))
    host_of = {"xb": xb, "wsb": wsb, "b2d": b2d, "brd": brd, "oned": oned,
               "idd": idd}
    spec_of = {n: repl for n in ("wsb", "b2d", "brd", "oned", "idd")}
    staged = [jax.device_put(host_of[n], spec_of.get(n, shard))
              for n in in_names if n != "dum"]
    jax.block_until_ready(staged)

    mkout = jax.jit(
        lambda: jnp.zeros((NC * 2 * N8, D), jnp.bfloat16),
        out_shardings=shard)
    return staged, mkout, inv


def run_device(staged, oi, repeat=1, parts="full"):
    ent = _get_callable(repeat, parts)
    fn, dum = ent[0], ent[4]
    return fn(dum, *staged, oi)[0]


def run_chain(staged, oi, n, repeat=1, parts="full"):
    ent = _get_callable(repeat, parts)
    fn, dum = ent[0], ent[4]
    r = oi
    for _ in range(n):
        r = fn(dum, *staged, r)[0]
    r.block_until_ready()
    return r


def kernel(x, W, b, arg_idx, target_idx):
    staged, mkout, _ = prepare(x, W, b, arg_idx, target_idx)
    res = run_device(staged, mkout(), parts=BEST)
    res = np.asarray(res)
    return assemble(res, np.asarray(arg_idx, dtype=np.int64),
                    np.asarray(target_idx, dtype=np.int64))


# revision 23
# speedup vs baseline: 1.0099x; 1.0099x over previous
"""Dense-recompute distributed Trainium2 kernel for:
    out = x.at[target_idx].set(relu(x[arg_idx] @ W + b))

N=2097152 rows x D=64, K=1048576 gathered/scattered rows, 8 NeuronCores.

Strategy v3 (vs the v2 gather kernel at ~1.38 ms): random-row gathers are
descriptor-rate-bound (~9 ns/row/core), so instead every core streams its own
contiguous 33.5 MB slice of a pre-staged bf16 copy of x ONCE and computes
f(row) = relu(row @ W + b) for ALL of its rows densely - sequential DMA only:

  per 16384-row chunk ([128, 8192] bf16 tile, partition-major):
    - load on the sync queue; pass-through store (unchanged bytes) on the
      scalar queue
    - PE-transposes 128-col slices into PSUM (feat-on-partition, two windows
      stacked), DVE copies PSUM->SBUF, ONE K=128 matmul per 512 rows against
      the block-diagonal [[W,0],[0,W]] (operands must be partition-0 based:
      offset-64 matmul operands wedge this hardware), ACT relu(+bias) writes
      the bf16 flush tile directly in y^T layout (no back-transpose)
    - F flush store alternates sync/scalar queues

Outputs per core: [F stream y^T-layout; pass-through row-major], both bf16
(gate is rel_err < 2e-2; bf16 lands ~2e-3). All output bytes are produced on
device; the host only routes: pass-through rows via an identity row gather,
target rows via a vectorized element gather from the y^T layout.

Per-core traffic 100.5 MB at the measured ~390 GB/s/core. Fully DMA-bound:
a pure-DMA ablation of the same 3-stream pattern (parts='pt3') times within
~8 us of the full kernel, so the whole compute pipeline hides under the DMA.
Measured on 8 axon-tunneled trn2 cores via repeat-16-in-NEFF chain slope
(warmup pair discarded): ~310-320 us/exec, rel err 2.14e-3, vs the 1.48 ms
v2 gather baseline -- ~4.7x. Adopted: '_f2' split F stores + '_io' per-chunk
queue interleave (+6 us in head-to-head). Rejected by measurement: finer chunks (_ch8), queue
interleave (_io), gpsimd F-stores (_fg), direct DRAM->DRAM pass-through
(_pd, 100 us WORSE: few-descriptor D2D parallelizes poorly), F compaction
(one-hot mask gen on DVE costs ~350 us to save ~60 us of DMA), fp8 F stream
(norm rel err ~2.5e-2 exceeds the 2e-2 gate).
"""

import numpy as np
import ml_dtypes

import jax
import jax.numpy as jnp
from jax.sharding import Mesh, PartitionSpec, NamedSharding
from jax.experimental.shard_map import shard_map

import concourse.bass as bass
import concourse.bacc as bacc
import concourse.mybir as mybir
import concourse.bass2jax as bass2jax
from concourse.tile import TileContext

bf16 = ml_dtypes.bfloat16

N = 2097152
D = 64
K = 1048576
NC = 8
N8 = N // NC          # 262144 rows per core
CH = 16384            # rows per chunk
C = CH // 128         # 128 col-blocks (rows per partition) per chunk
NCH = N8 // CH        # 16 chunks per core
NG = C // 8           # 16 groups of 8 col-blocks (512 cols) per chunk

_CAP = 1  # walrus: one semaphore wait per instruction


def _split_excess_waits(nc):
    for f in nc.m.functions:
        for bb in f.blocks:
            insts = list(bb.instructions)
            out = []
            changed = False
            for inst in insts:
                si = inst.sync_info
                if si is not None and len(si.on_wait) > _CAP:
                    waits = list(si.on_wait)
                    head, tail = waits[:-_CAP], waits[-_CAP:]
                    for i in range(0, len(head), _CAP):
                        nop = mybir.InstNoOp(
                            name=f"waitsplit_{nc.next_id()}", ins=[], outs=[])
                        nop.engine = inst.engine
                        nop.sync_info = mybir.SyncInfo(
                            on_wait=head[i:i + _CAP], on_update=[])
                        out.append(nop)
                    si.on_wait = tail
                    inst.sync_info = si
                    changed = True
                out.append(inst)
            if changed:
                bb.instructions = out


PARTS = ["full", "pt", "comp", "engines", "comp1", "comp2", "empty", "pt3"]
BUILD_SALT = 2  # bump on any kernel change: the NEFF cache hashes HLO shapes,
                # not the embedded BIR, so same-shape rebuilds go stale


def build_nc(repeat=1, parts="full", split_waits=True):
    """parts base: 'full' | 'pt' | 'comp' (no PT store) | 'engines' (1 load,
    all compute) | 'comp1' (transpose+copy only) | 'comp2' (no back-transpose).
    Suffixes: '_fg' F stores on gpsimd (SWDGE) queue; '_bm' bias via K=1 matmul
    instead of ACT bias AP."""
    f_gp = "_fg" in parts
    bias_mm = "_bm" in parts
    b4 = "_b4" in parts   # deeper pool buffering
    b5 = "_b5" in parts   # even deeper pool buffering
    l2 = "_l2" in parts   # split chunk loads in halves (earlier group release)
    c2 = "_c2" in parts   # split psum->sbuf copies in halves for latency hiding
    io = "_io" in parts   # interleave load/PT/F queue assignment per chunk
    pd = "_pd" in parts   # pass-through as direct DRAM->DRAM (no SBUF dep)
    f2 = "_f2" in parts   # F store in two halves (earlier queue feed)
    ch = CH // 2 if "_ch8" in parts else (CH * 2 if "_ch32" in parts else CH)
    c_w = ch // 128       # col-blocks per chunk
    ng = c_w // 8         # 512-col groups per chunk
    nch = N8 // ch
    nc = bacc.Bacc(num_swdge_queues=1)
    nc.declare_dram_parameter(
        "dum", [1, 1 + BUILD_SALT * 16 + PARTS.index(parts.split("_")[0])],
        mybir.dt.float32, isOutput=False)
    xb = nc.declare_dram_parameter("xb", [N8, D], mybir.dt.bfloat16, isOutput=False)
    wsb = nc.declare_dram_parameter("wsb", [128, 128], mybir.dt.bfloat16, isOutput=False)
    b2d = nc.declare_dram_parameter("b2d", [128, 1], mybir.dt.float32, isOutput=False)
    brd = nc.declare_dram_parameter("brd", [1, 128], mybir.dt.bfloat16, isOutput=False)
    oned = nc.declare_dram_parameter("oned", [1, 512], mybir.dt.bfloat16, isOutput=False)
    idd = nc.declare_dram_parameter("idd", [128, 128], mybir.dt.bfloat16, isOutput=False)
    out = nc.declare_dram_parameter("out", [2 * N8, D], mybir.dt.bfloat16, isOutput=True)

    base = parts.split("_")[0]
    do_pt = base in ("full", "pt", "pt3")
    do_comp = base not in ("pt", "pt3", "empty")
    one_load = base == "engines"
    yt_mode = "_yt" in parts  # F stays in y^T layout; no back-transposes
    # comp1: stop after transpose+copy (store xts); comp2: stop after relu
    # (store yt); 3 = full pipeline
    stage = {"comp1": 1, "comp2": 2}.get(base, 3)

    with TileContext(nc) as tc:
        with (
            tc.tile_pool(name="wt", bufs=1) as wpool,
            tc.tile_pool(name="x", bufs=4 if b5 else (3 if b4 else 2)) as xpool,
            tc.tile_pool(name="xt", bufs=5 if b5 else (4 if b4 else 3)) as xtpool,
            tc.tile_pool(name="yt", bufs=3) as ytpool,
            tc.tile_pool(name="ft", bufs=3 if (b4 or b5) else 2) as fpool,
            tc.tile_pool(name="pT", bufs=4 if (b4 or b5) else 2, space="PSUM") as pTpool,
            tc.tile_pool(name="pY", bufs=4 if b5 else (3 if b4 else 2), space="PSUM") as pYpool,
            tc.tile_pool(name="pR", bufs=2, space="PSUM") as pRpool,
        ):
            wt = wpool.tile([128, 128], mybir.dt.bfloat16, tag="wt")
            bt = wpool.tile([128, 1], mybir.dt.float32, tag="bt")
            brt = wpool.tile([1, 128], mybir.dt.bfloat16, tag="brt")
            onet = wpool.tile([1, 512], mybir.dt.bfloat16, tag="onet")
            idt = wpool.tile([128, 128], mybir.dt.bfloat16, tag="idt")
            nc.sync.dma_start(out=wt[:], in_=wsb[:, :])
            nc.sync.dma_start(out=bt[:], in_=b2d[:, :])
            nc.sync.dma_start(out=brt[:], in_=brd[:, :])
            nc.sync.dma_start(out=onet[:], in_=oned[:, :])
            nc.sync.dma_start(out=idt[:], in_=idd[:, :])

            for rep in range(repeat):
                if base == "empty":
                    et = xpool.tile([128, D], mybir.dt.bfloat16, tag="e",
                                    name=f"e_{rep}")
                    nc.sync.dma_start(
                        out=et[:],
                        in_=xb[0:128, :].rearrange("(p c) d -> p (c d)", p=128))
                    nc.scalar.dma_start(
                        out=out[0:128, :].rearrange("(p c) d -> p (c d)", p=128),
                        in_=et[:])
                    continue
                for c in range(nch):
                    s = c * ch
                    ld_eng = (nc.sync if (not io or c % 2 == 0) else nc.scalar)
                    pt_eng = (nc.scalar if (not io or c % 2 == 0) else nc.sync)
                    xt_ = xpool.tile([128, c_w * D], mybir.dt.bfloat16, tag="x",
                                     name=f"x_{rep}_{c}")
                    if not one_load or c == 0:
                        lap = xb[s:s + ch, :].rearrange("(p c) d -> p (c d)", p=128)
                        if l2:
                            lh = c_w * D // 2
                            ld_eng.dma_start(out=xt_[:, 0:lh], in_=lap[:, 0:lh])
                            ld_eng.dma_start(out=xt_[:, lh:], in_=lap[:, lh:])
                        else:
                            ld_eng.dma_start(out=xt_[:], in_=lap)
                    if do_pt:
                        if pd:
                            # direct DRAM->DRAM identity copy: same bytes, but
                            # independent of the SBUF load
                            pt_eng.dma_start(
                                out=out[N8 + s:N8 + s + ch, :],
                                in_=xb[s:s + ch, :])
                        else:
                            pt_eng.dma_start(
                                out=out[N8 + s:N8 + s + ch, :].rearrange(
                                    "(p c) d -> p (c d)", p=128),
                                in_=xt_[:])
                    if base == "pt3":
                        feng = nc.sync if c % 2 == 0 else nc.scalar
                        feng.dma_start(
                            out=out[s:s + ch, :].rearrange("(p c) d -> p (c d)", p=128),
                            in_=xt_[:])
                        continue
                    if not do_comp:
                        continue
                    ft = fpool.tile([128, c_w * D], mybir.dt.bfloat16, tag="ft",
                                    name=f"ft_{rep}_{c}")
                    for g in range(ng):
                        c0 = g * 512
                        pT = pTpool.tile([128, 512], mybir.dt.bfloat16, tag="pT",
                                         name=f"pT_{rep}_{c}_{g}")
                        for q in range(4):
                            nc.tensor.transpose(
                                pT[:, q * 128:(q + 1) * 128],
                                xt_[:, c0 + q * 128:c0 + (q + 1) * 128],
                                idt[:])
                        xts = xtpool.tile([128, 512], mybir.dt.bfloat16, tag="xts",
                                          name=f"xts_{rep}_{c}_{g}")
                        if c2:
                            nc.vector.tensor_copy(out=xts[:, 0:256], in_=pT[:, 0:256])
                            nc.vector.tensor_copy(out=xts[:, 256:512], in_=pT[:, 256:512])
                        else:
                            nc.vector.tensor_copy(out=xts[:], in_=pT[:])
                        if stage == 1:
                            nc.vector.tensor_copy(out=ft[:, c0:c0 + 512], in_=xts[:])
                            continue
                        pY = pYpool.tile([128, 512], mybir.dt.float32, tag="pY",
                                         name=f"pY_{rep}_{c}_{g}")
                        # single K=128 matmul: wt is block-diag [[W,0],[0,W]],
                        # so both window halves compute at once with all
                        # operands partition-0 based (offset-64 operands wedge
                        # this hardware)
                        if bias_mm:
                            nc.tensor.matmul(pY[:], brt[0:1, :], onet[0:1, :],
                                             start=True, stop=False)
                            nc.tensor.matmul(pY[:], wt[:, :], xts[:, :],
                                             start=False, stop=True)
                        else:
                            nc.tensor.matmul(pY[:], wt[:, :], xts[:, :],
                                             start=True, stop=True)
                        relu_kw = {} if bias_mm else {"bias": bt[:, 0:1]}
                        if yt_mode:
                            # relu writes the flush tile directly (y^T layout)
                            nc.scalar.activation(
                                ft[:, c0:c0 + 512], pY[:],
                                mybir.ActivationFunctionType.Relu, **relu_kw)
                            continue
                        yt = ytpool.tile([128, 512], mybir.dt.bfloat16, tag="yt",
                                         name=f"yt_{rep}_{c}_{g}")
                        nc.scalar.activation(
                            yt[:], pY[:], mybir.ActivationFunctionType.Relu,
                            **relu_kw)
                        if stage == 2:
                            nc.vector.tensor_copy(out=ft[:, c0:c0 + 512], in_=yt[:])
                            continue
                        pR = pRpool.tile([128, 512], mybir.dt.bfloat16, tag="pR",
                                         name=f"pR_{rep}_{c}_{g}")
                        for q in range(4):
                            nc.tensor.transpose(
                                pR[:, q * 128:(q + 1) * 128],
                                yt[:, q * 128:(q + 1) * 128],
                                idt[:])
                        nc.vector.tensor_copy(out=ft[:, c0:c0 + 512], in_=pR[:])
                    # F store: gpsimd queue, or alternate sync/scalar
                    if f_gp:
                        feng = nc.gpsimd
                    else:
                        feng = nc.sync if c % 2 == 0 else nc.scalar
                    fap = out[s:s + ch, :].rearrange("(p c) d -> p (c d)", p=128)
                    if f2:
                        half = c_w * D // 2
                        feng.dma_start(out=fap[:, 0:half], in_=ft[:, 0:half])
                        feng.dma_start(out=fap[:, half:], in_=ft[:, half:])
                    else:
                        feng.dma_start(out=fap, in_=ft[:])
    nc.compile()
    if split_waits:
        _split_excess_waits(nc)
    return nc


_CACHE = {}


def _get_callable(repeat=1, parts="full"):
    key = f"fn_{repeat}_{parts}"
    if key in _CACHE:
        return _CACHE[key]
    bass2jax.install_neuronx_cc_hook()
    nc = build_nc(repeat, parts)

    pname = nc.partition_id_tensor.name if nc.partition_id_tensor else None
    in_names, out_names, out_avals = [], [], []
    for alloc in nc.m.functions[0].allocations:
        if not isinstance(alloc, mybir.MemoryLocationSet):
            continue
        name = alloc.memorylocations[0].name
        if alloc.kind == "ExternalInput":
            if name != pname:
                in_names.append(name)
        elif alloc.kind == "ExternalOutput":
            out_names.append(name)
            out_avals.append(
                jax.core.ShapedArray(tuple(alloc.tensor_shape), mybir.dt.np(alloc.dtype)))
    assert in_names[0] == "dum", in_names
    dum_shape = None
    for alloc in nc.m.functions[0].allocations:
        if (isinstance(alloc, mybir.MemoryLocationSet)
                and alloc.memorylocations[0].name == "dum"):
            dum_shape = tuple(alloc.tensor_shape)
    n_params = len(in_names)
    all_in = list(in_names) + list(out_names)
    if pname is not None:
        all_in.append(pname)

    def _body(*args):
        operands = list(args)
        if pname is not None:
            operands.append(bass2jax.partition_id_tensor())
        outs = bass2jax._bass_exec_p.bind(
            *operands,
            out_avals=tuple(out_avals),
            in_names=tuple(all_in),
            out_names=tuple(out_names),
            lowering_input_output_aliases=(),
            sim_require_finite=True,
            sim_require_nnan=True,
            nc=nc)
        return tuple(outs)

    devices = jax.devices()[:NC]
    mesh = Mesh(np.asarray(devices), ("core",))
    repl_names = {"wsb", "b2d", "brd", "oned", "idd", "dum"}
    in_specs = tuple(
        PartitionSpec(None) if n in repl_names else PartitionSpec("core")
        for n in in_names) + (PartitionSpec("core"),) * len(out_names)
    out_specs = (PartitionSpec("core"),) * len(out_names)
    fn = jax.jit(
        shard_map(_body, mesh=mesh, in_specs=in_specs, out_specs=out_specs,
                  check_rep=False),
        donate_argnums=tuple(range(n_params, n_params + len(out_names))),
        keep_unused=True)
    dum = jax.device_put(np.zeros(dum_shape, np.float32),
                         NamedSharding(mesh, PartitionSpec(None)))
    _CACHE[key] = (fn, in_names, out_names, mesh, dum)
    return _CACHE[key]


BEST = "full_yt_b4_f2_io"  # parts string used by kernel() and the test harness


def _assemble_rowmajor(res, arg, tgt):
    i = np.arange(N, dtype=np.int64)
    inv = (i // N8) * (2 * N8) + N8 + (i % N8)
    inv[tgt] = (arg // N8) * (2 * N8) + (arg % N8)
    return res[inv].astype(np.float32)


def _assemble_yt(res, arg, tgt):
    """F region is y^T-layout: feat f of in-chunk row rr=(p2*?; see build) lives
    at ft[64*(j%2)+f, (j//8)*512 + ((j%8)//2)*128 + p2], j=rr%128, p2=rr//128."""
    i = np.arange(N, dtype=np.int64)
    inv0 = (i // N8) * (2 * N8) + N8 + (i % N8)
    out = res[inv0].astype(np.float32)
    a = arg
    core = a // N8
    rr0 = a % N8
    c = rr0 // CH
    rr = rr0 % CH
    p2 = rr // C
    j = rr % C
    u = (j // 8) * 512 + ((j % 8) // 2) * 128 + p2
    m0 = 64 * (j % 2)
    base = core * (2 * N8) + c * CH + m0 * C + (u // 64)
    row_idx = base[:, None] + (np.arange(64, dtype=np.int64) * C)[None, :]
    col_idx = np.broadcast_to((u % 64)[:, None], row_idx.shape)
    vals = res[row_idx, col_idx].astype(np.float32)
    out[tgt] = vals
    return out


def assemble(res, arg, tgt, parts=None):
    parts = BEST if parts is None else parts
    if "_yt" in parts:
        return _assemble_yt(res, arg, tgt)
    return _assemble_rowmajor(res, arg, tgt)


def prepare(x, W, b, arg_idx, target_idx):
    x = np.asarray(x, dtype=np.float32)
    W = np.asarray(W, dtype=np.float32)
    b = np.asarray(b, dtype=np.float32)
    arg = np.asarray(arg_idx, dtype=np.int64)
    tgt = np.asarray(target_idx, dtype=np.int64)

    xb = x.astype(bf16)
    Wb = W.astype(bf16)
    wsb = np.zeros((128, 128), dtype=bf16)
    wsb[0:64, 0:64] = Wb
    wsb[64:128, 64:128] = Wb
    b2d = np.concatenate([b, b]).reshape(128, 1).astype(np.float32)
    brd = np.concatenate([b, b]).reshape(1, 128).astype(bf16)
    oned = np.ones((1, 512), dtype=bf16)
    idd = np.eye(128, dtype=bf16)

    # inverse map: PT region (identity) for everyone, F region for targets
    i = np.arange(N, dtype=np.int64)
    inv = (i // N8) * (2 * N8) + N8 + (i % N8)
    inv[tgt] = (arg // N8) * (2 * N8) + (arg % N8)

    fn, in_names, out_names, mesh, dum = _get_callable(1, BEST)
    repl = NamedSharding(mesh, PartitionSpec(None))
    shard = NamedSharding(mesh, PartitionSpec("core"))
    host_of = {"xb": xb, "wsb": wsb, "b2d": b2d, "brd": brd, "oned": oned,
               "idd": idd}
    spec_of = {n: repl for n in ("wsb", "b2d", "brd", "oned", "idd")}
    staged = [jax.device_put(host_of[n], spec_of.get(n, shard))
              for n in in_names if n != "dum"]
    jax.block_until_ready(staged)

    mkout = jax.jit(
        lambda: jnp.zeros((NC * 2 * N8, D), jnp.bfloat16),
        out_shardings=shard)
    return staged, mkout, inv


def run_device(staged, oi, repeat=1, parts="full"):
    ent = _get_callable(repeat, parts)
    fn, dum = ent[0], ent[4]
    return fn(dum, *staged, oi)[0]


def run_chain(staged, oi, n, repeat=1, parts="full"):
    ent = _get_callable(repeat, parts)
    fn, dum = ent[0], ent[4]
    r = oi
    for _ in range(n):
        r = fn(dum, *staged, r)[0]
    r.block_until_ready()
    return r


def kernel(x, W, b, arg_idx, target_idx):
    staged, mkout, _ = prepare(x, W, b, arg_idx, target_idx)
    res = run_device(staged, mkout(), parts=BEST)
    res = np.asarray(res)
    return assemble(res, np.asarray(arg_idx, dtype=np.int64),
                    np.asarray(target_idx, dtype=np.int64))
